# revision 5
# baseline (speedup 1.0000x reference)
"""APKDA loss (pool+normalize -> SmoothAP) as a distributed Bass kernel on 8 TRN2 NeuronCores.

Math restructuring vs the reference:
  - Only the diagonal class-blocks of sim_all_rk are ever used, so per query q we
    need rank sums only over its 16 same-class columns j:
        r_all[q,j] = 1 + sum_k relu(S[q,k] - S[q,j])   (k over all 512 columns)
        r_pos[q,j] = 1 + sum_k relu(Sg[q,k] - Sg[q,j]) (k over the 16-group)
    with Sg the own-class Gram block.  The eye-masks in the reference only kill
    k==j terms whose relu is 0 anyway.
  - L2-normalizing the hw-sum equals normalizing the hw-mean (scale cancels).

Sharding: batch-parallel.  Core m owns 4 classes = outputs[32m:32m+32] and
targets[32m:32m+32] (6.4MB of the 51.4MB input).  It pools + normalizes its 64
f-rows into a [512, 64] f^T block, AllGathers the blocks (128KB/rank), computes
its [64, 512] slice of S, the rank sums, and a partial sum of r_pos/r_all.  The
host sums the 8 partial scalars: loss = 1 - total/(16*512).
"""

import numpy as np

import concourse.bass as bass
import concourse.bacc as bacc
import concourse.mybir as mybir
import concourse.tile as tile
from concourse.bass_utils import run_bass_kernel_spmd

F32 = mybir.dt.float32
NCORES = 8
BATCH, FEAT, HW = 256, 512, 49
BPC = BATCH // NCORES          # 32 batch rows per branch per core
GROUP, B2 = 16, 512
CH = 1568                      # free-dim chunk for pooling pipeline (32 c * 49)

# j-slot split of the 8 (two-j-per-instruction) rank iterations
DVE_SLOTS = 5                  # slots 0..4 on VectorE, 5..7 on ScalarE


def build_kernel_body(nc, tc, x_out, x_tgt, out_d, ident_d, mbsum_d, ones_d,
                      dbg=None):
    f32 = F32
    AF = mybir.ActivationFunctionType
    ALU = mybir.AluOpType
    AX = mybir.AxisListType

    with (
        tc.tile_pool(name="sb", bufs=1) as sb,
        tc.tile_pool(name="ps", bufs=1, space="PSUM") as ps,
        tc.tile_pool(name="dr", bufs=1, space="DRAM") as dr,
    ):
        # ---- constants ----
        ident = sb.tile([128, 128], f32, tag="ident")
        mbsum = sb.tile([128, 128], f32, tag="mbsum")
        onesc = sb.tile([128, 1], f32, tag="onesc")
        nc.sync.dma_start(ident[:, :], ident_d.ap())
        nc.sync.dma_start(mbsum[:, :], mbsum_d.ap())
        nc.sync.dma_start(onesc[:, :], ones_d.ap())

        # ---- input tiles: partition = (g, b) with g = c-chunk of 128, b = 0..31 ----
        xo = sb.tile([128, 4 * CH], f32, tag="xo")
        xt = sb.tile([128, 4 * CH], f32, tag="xt")
        pooled_o = sb.tile([128, 128], f32, tag="pooled_o")
        pooled_t = sb.tile([128, 128], f32, tag="pooled_t")

        # interleave chunk DMAs of both tensors so pooling can start early;
        # chunk (g, cc) = partitions [32g, 32g+32), free [CH*cc, CH*(cc+1))
        for cc in range(4):
            for g in range(4):
                c0 = g * 128 + cc * 32
                nc.sync.dma_start(
                    xo[32 * g:32 * (g + 1), CH * cc:CH * (cc + 1)],
                    x_out.ap()[:, c0:c0 + 32, :],
                )
                nc.sync.dma_start(
                    xt[32 * g:32 * (g + 1), CH * cc:CH * (cc + 1)],
                    x_tgt.ap()[:, c0:c0 + 32, :],
                )
            # pooling: reduce hw (innermost 49) for this chunk, both branches
            nc.vector.reduce_sum(
                pooled_o[:, 32 * cc:32 * (cc + 1)],
                xo[:, CH * cc:CH * (cc + 1)].rearrange("p (c h) -> p c h", h=HW),
                axis=AX.X,
            )
            nc.vector.reduce_sum(
                pooled_t[:, 32 * cc:32 * (cc + 1)],
                xt[:, CH * cc:CH * (cc + 1)].rearrange("p (c h) -> p c h", h=HW),
                axis=AX.X,
            )

        # ---- norms: r2[(g,b), branch] = sum_c pooled^2 over this chunk's 128 c ----
        scrap_n = sb.tile([128, 128], f32, tag="scrap_n")
        r2 = sb.tile([128, 2], f32, tag="r2")
        nc.scalar.activation(scrap_n[:, :], pooled_o[:, :], AF.Square,
                             accum_out=r2[:, 0:1])
        nc.scalar.activation(scrap_n[:, :], pooled_t[:, :], AF.Square,
                             accum_out=r2[:, 1:2])
        # sum the 4 g-groups per b (cross-partition) via 0/1 matmul; result
        # lands replicated per (g,b)
        ps_n = ps.tile([128, 2], f32, tag="ps_n")
        nc.tensor.matmul(ps_n[:, :], mbsum[:, :], r2[:, :], start=True, stop=True)
        nrm = sb.tile([128, 2], f32, tag="nrm")
        inv = sb.tile([128, 2], f32, tag="inv")
        nc.scalar.activation(nrm[:, :], ps_n[:, :], AF.Sqrt)
        nc.vector.reciprocal(inv[:, :], nrm[:, :])

        # ---- normalize + transpose to [c_local, (g, b)] ----
        fo_n = sb.tile([128, 128], f32, tag="fo_n")
        ft_n = sb.tile([128, 128], f32, tag="ft_n")
        nc.vector.tensor_scalar_mul(fo_n[:, :], pooled_o[:, :], inv[:, 0:1])
        nc.vector.tensor_scalar_mul(ft_n[:, :], pooled_t[:, :], inv[:, 1:2])
        psT_o = ps.tile([128, 128], f32, tag="psT_o")
        psT_t = ps.tile([128, 128], f32, tag="psT_t")
        nc.tensor.transpose(psT_o[:, :], fo_n[:, :], ident[:, :])
        nc.tensor.transpose(psT_t[:, :], ft_n[:, :], ident[:, :])

        # ---- own f^T block in SBUF, reference row order within the core:
        #      col = 16*class + 8*branch + i  (class = b//8, i = b%8) ----
        ccin = [sb.tile([128, 64], f32, tag=f"ccin{g}", name=f"ccin{g}")
                for g in range(4)]
        for g in range(4):
            dst_o = ccin[g][:, :].rearrange("p (c u) -> p c u", c=4)[:, :, 0:8]
            dst_t = ccin[g][:, :].rearrange("p (c u) -> p c u", c=4)[:, :, 8:16]
            src_o = psT_o[:, 32 * g:32 * (g + 1)].rearrange("p (c u) -> p c u", c=4)
            src_t = psT_t[:, 32 * g:32 * (g + 1)].rearrange("p (c u) -> p c u", c=4)
            nc.vector.tensor_copy(dst_o, src_o)
            nc.scalar.copy(dst_t, src_t)

        # ---- stage own block to DRAM and AllGather ----
        cc_in = dr.tile([512, 64], f32, tag="cc_in")
        cc_out = dr.tile([NCORES * 512, 64], f32, tag="cc_out")
        for g in range(4):
            nc.sync.dma_start(cc_in[128 * g:128 * (g + 1), :], ccin[g][:, :])
        nc.gpsimd.collective_compute(
            "AllGather",
            ALU.bypass,
            replica_groups=[list(range(NCORES))],
            ins=[cc_in[:, :].opt()],
            outs=[cc_out[:, :].opt()],
        )

        # ---- own-class Gram: G = sum_g ccin_g^T ccin_g  -> [64, 64] ----
        ps_G = ps.tile([64, 64], f32, tag="ps_G")
        for g in range(4):
            nc.tensor.matmul(ps_G[:, :], ccin[g][:, :], ccin[g][:, :],
                             start=(g == 0), stop=(g == 3))
        # compute engines need 32-aligned partition bases, so stage the Gram in
        # SBUF and pull the 16x16 diagonal blocks out with small DMAs instead
        G_sb = sb.tile([64, 64], f32, tag="G_sb")
        nc.vector.tensor_copy(G_sb[:, :], ps_G[:, :])
        Sg = sb.tile([64, 16], f32, tag="Sg")
        for c in range(4):
            nc.sync.dma_start(Sg[16 * c:16 * (c + 1), :],
                              G_sb[16 * c:16 * (c + 1), 16 * c:16 * (c + 1)])
        negSg = sb.tile([64, 16], f32, tag="negSg")
        nc.vector.tensor_scalar_mul(negSg[:, :], Sg[:, :], -1.0)

        # bias tile: B8[p, i] = -Sg[q, j] with (q, j) = (p, i) for p<64,
        # (p-64, i+8) for p>=64 ; SgD = Sg duplicated on both partition halves
        B8 = sb.tile([128, 8], f32, tag="B8")
        SgD = sb.tile([128, 16], f32, tag="SgD")
        nc.vector.tensor_copy(B8[0:64, :], negSg[:, 0:8])
        nc.vector.tensor_copy(B8[64:128, :], negSg[:, 8:16])
        nc.vector.tensor_copy(SgD[0:64, :], Sg[:, :])
        nc.vector.tensor_copy(SgD[64:128, :], Sg[:, :])

        # ---- rhs tiles from gathered f^T; column order is rank-major (any
        #      order is fine: k-sums are permutation invariant) ----
        rhs = [sb.tile([128, 512], f32, tag=f"rhs{g}", name=f"rhsg{g}")
               for g in range(4)]
        cc_view = cc_out[:, :].rearrange("(r g p) n -> g p r n", r=NCORES, g=4)
        for g in range(4):
            nc.sync.dma_start(
                rhs[g][:, :].rearrange("p (r n) -> p r n", r=NCORES),
                cc_view[g],
            )

        # ---- S slice: [64 own queries, 512 keys] ----
        ps_S = ps.tile([64, 512], f32, tag="ps_S")
        for g in range(4):
            nc.tensor.matmul(ps_S[:, :], ccin[g][:, :], rhs[g][:, :],
                             start=(g == 0), stop=(g == 3))

        # S duplicated on both partition halves for the 2-j-per-op rank loop
        Sb = sb.tile([128, 512], f32, tag="Sb")
        nc.scalar.copy(Sb[0:64, :], ps_S[:, :])
        nc.vector.tensor_copy(Sb[64:128, :], ps_S[:, :])

        # ---- rank sums ----
        zeros = sb.tile([128, 512], f32, tag="zeros")
        nc.vector.memset(zeros[:, :], 0.0)
        scrap_d = sb.tile([128, 512], f32, tag="scrap_d")
        scrap_a = sb.tile([128, 512], f32, tag="scrap_a")
        scrap_p = sb.tile([128, 16], f32, tag="scrap_p")
        racc_d = sb.tile([128, DVE_SLOTS], f32, tag="racc_d")
        racc_a = sb.tile([128, 8 - DVE_SLOTS], f32, tag="racc_a")
        racc_p = sb.tile([128, 8], f32, tag="racc_p")
        for i in range(8):
            if i < DVE_SLOTS:
                nc.vector.scalar_tensor_tensor(
                    out=scrap_d[:, :], in0=Sb[:, :], scalar=B8[:, i:i + 1],
                    in1=zeros[:, :], op0=ALU.add, op1=ALU.max,
                    accum_out=racc_d[:, i:i + 1])
            else:
                nc.scalar.activation(
                    scrap_a[:, :], Sb[:, :], AF.Relu, bias=B8[:, i:i + 1],
                    accum_out=racc_a[:, i - DVE_SLOTS:i - DVE_SLOTS + 1])
            nc.vector.scalar_tensor_tensor(
                out=scrap_p[:, :], in0=SgD[:, :], scalar=B8[:, i:i + 1],
                in1=zeros[:, 0:16], op0=ALU.add, op1=ALU.max,
                accum_out=racc_p[:, i:i + 1])

        if dbg:
            nc.sync.dma_start(dbg["dbg_pooled_o"].ap(), pooled_o[:, :])
            nc.sync.dma_start(dbg["dbg_pooled_t"].ap(), pooled_t[:, :])
            nc.sync.dma_start(dbg["dbg_inv"].ap(), inv[:, :])
            for g in range(4):
                nc.sync.dma_start(dbg["dbg_ccin"].ap()[128 * g:128 * (g + 1), :],
                                  ccin[g][:, :])
            nc.sync.dma_start(dbg["dbg_Sg"].ap(), Sg[:, :])
            nc.sync.dma_start(dbg["dbg_Sb"].ap(), Sb[:, :])
            nc.sync.dma_start(dbg["dbg_racc_d"].ap(), racc_d[:, :])
            nc.sync.dma_start(dbg["dbg_racc_a"].ap(), racc_a[:, :])
            nc.sync.dma_start(dbg["dbg_racc_p"].ap(), racc_p[:, :])

        # ---- partial AP sum: sum over (q, j) of (1+r_pos)/(1+r_all) ----
        ra1 = sb.tile([128, 8], f32, tag="ra1")
        rainv = sb.tile([128, 8], f32, tag="rainv")
        rp1 = sb.tile([128, 8], f32, tag="rp1")
        tq = sb.tile([128, 8], f32, tag="tq")
        tsum = sb.tile([128, 1], f32, tag="tsum")
        nc.vector.tensor_scalar_add(ra1[:, 0:DVE_SLOTS], racc_d[:, :], 1.0)
        nc.vector.tensor_scalar_add(ra1[:, DVE_SLOTS:8], racc_a[:, :], 1.0)
        nc.vector.reciprocal(rainv[:, :], ra1[:, :])
        nc.vector.tensor_scalar_add(rp1[:, :], racc_p[:, :], 1.0)
        nc.vector.tensor_tensor(out=tq[:, :], in0=rp1[:, :], in1=rainv[:, :],
                                op=ALU.mult)
        nc.vector.reduce_sum(tsum[:, :], tq[:, :], axis=AX.X)
        ps_tot = ps.tile([1, 1], f32, tag="ps_tot")
        nc.tensor.matmul(ps_tot[:, :], onesc[:, :], tsum[:, :],
                         start=True, stop=True)
        out_sb = sb.tile([1, 1], f32, tag="out_sb")
        nc.scalar.copy(out_sb[:, :], ps_tot[:, :])
        nc.sync.dma_start(out_d.ap(), out_sb[:, :])


def build_nc(debug_outputs=False):
    nc = bacc.Bacc("TRN2", target_bir_lowering=False, debug=False,
                   num_devices=NCORES)
    x_out = nc.dram_tensor("x_out", [BPC, FEAT, HW], F32, kind="ExternalInput")
    x_tgt = nc.dram_tensor("x_tgt", [BPC, FEAT, HW], F32, kind="ExternalInput")
    out_d = nc.dram_tensor("out", [1, 1], F32, kind="ExternalOutput")
    ident_d = nc.inline_tensor(np.eye(128, dtype=np.float32), "ident_c")
    mbsum_d = nc.inline_tensor(
        np.tile(np.eye(32, dtype=np.float32), (4, 4)), "mbsum_c")
    ones_d = nc.inline_tensor(np.ones((128, 1), dtype=np.float32), "ones_c")
    dbg = {}
    if debug_outputs:
        for name, shape in [("dbg_pooled_o", [128, 128]), ("dbg_pooled_t", [128, 128]),
                            ("dbg_inv", [128, 2]), ("dbg_ccin", [512, 64]),
                            ("dbg_Sg", [64, 16]), ("dbg_Sb", [128, 512]),
                            ("dbg_racc_d", [128, DVE_SLOTS]),
                            ("dbg_racc_a", [128, 8 - DVE_SLOTS]),
                            ("dbg_racc_p", [128, 8])]:
            dbg[name] = nc.dram_tensor(name, shape, F32, kind="ExternalOutput")
    with tile.TileContext(nc) as tc:
        build_kernel_body(nc, tc, x_out, x_tgt, out_d, ident_d, mbsum_d, ones_d,
                          dbg=dbg)
    nc.compile()
    return nc


_NC = None


def _get_nc():
    global _NC
    if _NC is None:
        _NC = build_nc()
    return _NC


def make_in_maps(outputs, targets):
    outputs = np.ascontiguousarray(
        np.asarray(outputs, dtype=np.float32)).reshape(BATCH, FEAT, HW)
    targets = np.ascontiguousarray(
        np.asarray(targets, dtype=np.float32)).reshape(BATCH, FEAT, HW)
    return [
        {
            "x_out": np.ascontiguousarray(outputs[m * BPC:(m + 1) * BPC]),
            "x_tgt": np.ascontiguousarray(targets[m * BPC:(m + 1) * BPC]),
        }
        for m in range(NCORES)
    ]


def finish(results):
    total = sum(float(results[m]["out"][0, 0]) for m in range(NCORES))
    return np.array(1.0 - total / (GROUP * B2), dtype=np.float32)


def kernel(outputs, targets):
    nc = _get_nc()
    res = run_bass_kernel_spmd(nc, make_in_maps(outputs, targets),
                               core_ids=list(range(NCORES)))
    return finish(res.results)


if __name__ == "__main__":
    import reference as ref
    inputs = ref.setup_inputs()
    actual = kernel(**{k: np.asarray(v) for k, v in inputs.items()})
    print("kernel result:", actual)


# revision 6
# speedup vs baseline: 1.4207x; 1.4207x over previous
"""APKDA loss (pool+normalize -> SmoothAP) as two distributed Bass launches on
8 TRN2 NeuronCores.

Math restructuring vs the reference:
  - Only the diagonal class-blocks of sim_all_rk are ever used, so per query q
    we need rank sums only over its 16 same-class columns j:
        r_all[q,j] = 1 + sum_k relu(S[q,k] - S[q,j])   (k over all 512 columns)
        r_pos[q,j] = 1 + sum_k relu(Sg[q,k] - Sg[q,j]) (k over the 16-group)
    with Sg the own-class block of S.  The eye-masks in the reference only
    kill k==j terms whose relu is 0 anyway.
  - L2-normalizing the hw-sum equals normalizing the hw-mean (scale cancels).

Sharding: batch-parallel.  Core m owns 4 classes = outputs[32m:32m+32] and
targets[32m:32m+32] (6.4MB of the 51.4MB input).

Phase 1 (memory-bound): each core pools + normalizes its 64 f-rows and writes
its [4,128,64] f^T block (feature-major) plus the own-class Gram diag Sg.
Phase 2: host concatenates the 8 blocks into the full f^T (1MB), feeds it to
every core (plus that core's own block), and each core computes its [64, 512]
slice of S, the rank sums, and a partial sum of r_pos/r_all.  The host sums
the 8 partial scalars: loss = 1 - total/(16*512).

A single-launch variant with an in-kernel AllGather was measured at 133.7us:
collectives on this runtime have a ~40-80us latency floor regardless of size,
so the f exchange goes through the host instead (two NEFF launches at ~14us
fixed overhead each).
"""

import numpy as np

import concourse.bass as bass
import concourse.bacc as bacc
import concourse.mybir as mybir
import concourse.tile as tile
from concourse.bass_utils import run_bass_kernel_spmd

F32 = mybir.dt.float32
NCORES = 8
BATCH, FEAT, HW = 256, 512, 49
BPC = BATCH // NCORES          # 32 batch rows per branch per core
GROUP, B2 = 16, 512
CH = 1568                      # pooling free-dim chunk (32 c * 49)

# j-slot split of the 8 (two-j-per-instruction) rank iterations
DVE_SLOTS = 5                  # slots 0..4 on VectorE, 5..7 on ScalarE

# matmul dtype for the big S matmul in phase 2: float32 (exact, 4 cyc/row),
# float32r (TF32-ish, 1 cyc/row at N>=256) or bfloat16
S_MM_DTYPE = F32


def build_phase1(dbg=None):
    """Pool + normalize + transpose; out: fT [4,128,64] (d-major), sg [64,16]."""
    nc = bacc.Bacc("TRN2", target_bir_lowering=False, debug=False,
                   num_devices=NCORES)
    f32 = F32
    AF = mybir.ActivationFunctionType
    AX = mybir.AxisListType
    x_out = nc.dram_tensor("x_out", [BPC, FEAT, HW], f32, kind="ExternalInput")
    x_tgt = nc.dram_tensor("x_tgt", [BPC, FEAT, HW], f32, kind="ExternalInput")
    fT_d = nc.dram_tensor("fT", [4, 128, 64], f32, kind="ExternalOutput")
    sg_d = nc.dram_tensor("sg", [64, 16], f32, kind="ExternalOutput")
    ident_d = nc.inline_tensor(np.eye(128, dtype=np.float32), "ident_c")
    mbsum_d = nc.inline_tensor(
        np.tile(np.eye(32, dtype=np.float32), (4, 4)), "mbsum_c")

    with tile.TileContext(nc) as tc, (
            tc.tile_pool(name="sb", bufs=1)) as sb, (
            tc.tile_pool(name="ps", bufs=1, space="PSUM")) as ps:
        xo = sb.tile([128, 4 * CH], f32, tag="xo")
        xt = sb.tile([128, 4 * CH], f32, tag="xt")
        pooled_o = sb.tile([128, 128], f32, tag="pooled_o")
        pooled_t = sb.tile([128, 128], f32, tag="pooled_t")

        # Input DMAs.  partition p = 32g + b; row (g,b) holds x[b, 128g:128g+128, :]
        # flattened.  DMAs are [32, CH] quarters: partitions 0-63 map to the 8
        # even SDMA engines, 64-127 to the odd ones, so route g=0,1 via sync
        # and g=2,3 via scalar to keep all 16 engines busy; quarter chunks
        # let pooling start early.
        for cc in range(4):
            for g in range(4):
                c0 = g * 128 + cc * 32
                for t_, x_ in ((xo, x_out), (xt, x_tgt)):
                    eng = nc.sync if g < 2 else nc.scalar
                    eng.dma_start(
                        t_[32 * g:32 * (g + 1), CH * cc:CH * (cc + 1)],
                        x_.ap()[:, c0:c0 + 32, :])
            for t_, p_ in ((xo, pooled_o), (xt, pooled_t)):
                nc.vector.reduce_sum(
                    p_[:, 32 * cc:32 * (cc + 1)],
                    t_[:, CH * cc:CH * (cc + 1)].rearrange("p (c h) -> p c h", h=HW),
                    axis=AX.X)

        # constants (queued after the input DMAs so they don't delay them)
        ident = sb.tile([128, 128], f32, tag="ident")
        mbsum = sb.tile([128, 128], f32, tag="mbsum")
        nc.sync.dma_start(ident[:, :], ident_d.ap())
        nc.sync.dma_start(mbsum[:, :], mbsum_d.ap())

        # norms: r2[(g,b), branch] = sum_c pooled^2 over this chunk's 128 c
        scrap_n = sb.tile([128, 128], f32, tag="scrap_n")
        r2 = sb.tile([128, 2], f32, tag="r2")
        nc.scalar.activation(scrap_n[:, :], pooled_o[:, :], AF.Square,
                             accum_out=r2[:, 0:1])
        nc.scalar.activation(scrap_n[:, :], pooled_t[:, :], AF.Square,
                             accum_out=r2[:, 1:2])
        # sum the 4 g-groups per b (cross-partition) via 0/1 matmul -> replicated
        ps_n = ps.tile([128, 2], f32, tag="ps_n")
        nc.tensor.matmul(ps_n[:, :], mbsum[:, :], r2[:, :], start=True, stop=True)
        nrm = sb.tile([128, 2], f32, tag="nrm")
        inv = sb.tile([128, 2], f32, tag="inv")
        nc.scalar.activation(nrm[:, :], ps_n[:, :], AF.Sqrt)
        nc.vector.reciprocal(inv[:, :], nrm[:, :])

        # normalize + transpose to [c_local, (g, b)]
        fo_n = sb.tile([128, 128], f32, tag="fo_n")
        ft_n = sb.tile([128, 128], f32, tag="ft_n")
        nc.vector.tensor_scalar_mul(fo_n[:, :], pooled_o[:, :], inv[:, 0:1])
        nc.vector.tensor_scalar_mul(ft_n[:, :], pooled_t[:, :], inv[:, 1:2])
        psT_o = ps.tile([128, 128], f32, tag="psT_o")
        psT_t = ps.tile([128, 128], f32, tag="psT_t")
        nc.tensor.transpose(psT_o[:, :], fo_n[:, :], ident[:, :])
        nc.tensor.transpose(psT_t[:, :], ft_n[:, :], ident[:, :])

        # own f^T block, reference row order: col = 16*class + 8*branch + i
        ccin = [sb.tile([128, 64], f32, tag=f"ccin{g}", name=f"ccin{g}")
                for g in range(4)]
        for g in range(4):
            dst_o = ccin[g][:, :].rearrange("p (c u) -> p c u", c=4)[:, :, 0:8]
            dst_t = ccin[g][:, :].rearrange("p (c u) -> p c u", c=4)[:, :, 8:16]
            src_o = psT_o[:, 32 * g:32 * (g + 1)].rearrange("p (c u) -> p c u", c=4)
            src_t = psT_t[:, 32 * g:32 * (g + 1)].rearrange("p (c u) -> p c u", c=4)
            nc.vector.tensor_copy(dst_o, src_o)
            nc.scalar.copy(dst_t, src_t)
        for g in range(4):
            nc.sync.dma_start(fT_d.ap()[g], ccin[g][:, :])

        # own-class Gram: G = sum_g ccin_g^T ccin_g -> [64, 64]
        ps_G = ps.tile([64, 64], f32, tag="ps_G")
        for g in range(4):
            nc.tensor.matmul(ps_G[:, :], ccin[g][:, :], ccin[g][:, :],
                             start=(g == 0), stop=(g == 3))
        # compute engines need 32-aligned partition bases, so stage the Gram
        # in SBUF and pull the 16x16 diagonal blocks out with small DMAs
        G_sb = sb.tile([64, 64], f32, tag="G_sb")
        nc.vector.tensor_copy(G_sb[:, :], ps_G[:, :])
        Sg = sb.tile([64, 16], f32, tag="Sg")
        for c in range(4):
            nc.scalar.dma_start(Sg[16 * c:16 * (c + 1), :],
                                G_sb[16 * c:16 * (c + 1), 16 * c:16 * (c + 1)])
        nc.sync.dma_start(sg_d.ap(), Sg[:, :])
    nc.compile()
    return nc


def build_phase2(dbg=None):
    """S slice + rank sums from replicated f^T.
    in: fT_all [4,128,512], fT_own [4,128,64], sg [64,16]; out: [1,1] partial."""
    nc = bacc.Bacc("TRN2", target_bir_lowering=False, debug=False,
                   num_devices=NCORES)
    f32 = F32
    AF = mybir.ActivationFunctionType
    ALU = mybir.AluOpType
    AX = mybir.AxisListType
    fT_all = nc.dram_tensor("fT_all", [4, 128, 512], f32, kind="ExternalInput")
    fT_own = nc.dram_tensor("fT_own", [4, 128, 64], f32, kind="ExternalInput")
    sg_in = nc.dram_tensor("sg", [64, 16], f32, kind="ExternalInput")
    out_d = nc.dram_tensor("out", [1, 1], f32, kind="ExternalOutput")
    ones_d = nc.inline_tensor(np.ones((128, 1), dtype=np.float32), "ones_c")

    with tile.TileContext(nc) as tc, (
            tc.tile_pool(name="sb", bufs=1)) as sb, (
            tc.tile_pool(name="ps", bufs=1, space="PSUM")) as ps:
        mm_dt = S_MM_DTYPE
        rhs = [sb.tile([128, 512], f32, tag=f"rhs{g}", name=f"rhsg{g}")
               for g in range(4)]
        ccin = [sb.tile([128, 64], f32, tag=f"ccin{g}", name=f"ccin{g}")
                for g in range(4)]
        # split each [128, x] load into even/odd partition halves on the two
        # HWDGE rings so all 16 SDMA engines are used
        for g in range(4):
            nc.sync.dma_start(rhs[g][0:64, :], fT_all.ap()[g, 0:64, :])
            nc.scalar.dma_start(rhs[g][64:128, :], fT_all.ap()[g, 64:128, :])
        for g in range(4):
            nc.sync.dma_start(ccin[g][0:64, :], fT_own.ap()[g, 0:64, :])
            nc.scalar.dma_start(ccin[g][64:128, :], fT_own.ap()[g, 64:128, :])
        Sg = sb.tile([64, 16], f32, tag="Sg")
        nc.sync.dma_start(Sg[:, :], sg_in.ap())
        onesc = sb.tile([128, 1], f32, tag="onesc")
        nc.sync.dma_start(onesc[:, :], ones_d.ap())

        # S slice: [64 own queries, 512 keys]
        ps_S = ps.tile([64, 512], f32, tag="ps_S")
        for g in range(4):
            nc.tensor.matmul(ps_S[:, :],
                             ccin[g][:, :].bitcast(mm_dt),
                             rhs[g][:, :].bitcast(mm_dt),
                             start=(g == 0), stop=(g == 3))
        Sb = sb.tile([128, 512], f32, tag="Sb")
        nc.scalar.copy(Sb[0:64, :], ps_S[:, :])
        nc.vector.tensor_copy(Sb[64:128, :], ps_S[:, :])

        negSg = sb.tile([64, 16], f32, tag="negSg")
        nc.vector.tensor_scalar_mul(negSg[:, :], Sg[:, :], -1.0)
        B8 = sb.tile([128, 8], f32, tag="B8")
        SgD = sb.tile([128, 16], f32, tag="SgD")
        nc.vector.tensor_copy(B8[0:64, :], negSg[:, 0:8])
        nc.vector.tensor_copy(B8[64:128, :], negSg[:, 8:16])
        nc.vector.tensor_copy(SgD[0:64, :], Sg[:, :])
        nc.vector.tensor_copy(SgD[64:128, :], Sg[:, :])

        zeros = sb.tile([128, 512], f32, tag="zeros")
        nc.vector.memset(zeros[:, :], 0.0)
        scrap_d = sb.tile([128, 512], f32, tag="scrap_d")
        scrap_a = sb.tile([128, 512], f32, tag="scrap_a")
        scrap_p = sb.tile([128, 16], f32, tag="scrap_p")
        racc_d = sb.tile([128, DVE_SLOTS], f32, tag="racc_d")
        racc_a = sb.tile([128, 8 - DVE_SLOTS], f32, tag="racc_a")
        racc_p = sb.tile([128, 8], f32, tag="racc_p")
        for i in range(8):
            if i < DVE_SLOTS:
                nc.vector.scalar_tensor_tensor(
                    out=scrap_d[:, :], in0=Sb[:, :], scalar=B8[:, i:i + 1],
                    in1=zeros[:, :], op0=ALU.add, op1=ALU.max,
                    accum_out=racc_d[:, i:i + 1])
            else:
                nc.scalar.activation(
                    scrap_a[:, :], Sb[:, :], AF.Relu, bias=B8[:, i:i + 1],
                    accum_out=racc_a[:, i - DVE_SLOTS:i - DVE_SLOTS + 1])
            nc.vector.scalar_tensor_tensor(
                out=scrap_p[:, :], in0=SgD[:, :], scalar=B8[:, i:i + 1],
                in1=zeros[:, 0:16], op0=ALU.add, op1=ALU.max,
                accum_out=racc_p[:, i:i + 1])

        # partial AP sum: sum over (q, j) of (1+r_pos)/(1+r_all)
        ra1 = sb.tile([128, 8], f32, tag="ra1")
        rainv = sb.tile([128, 8], f32, tag="rainv")
        rp1 = sb.tile([128, 8], f32, tag="rp1")
        tq = sb.tile([128, 8], f32, tag="tq")
        tsum = sb.tile([128, 1], f32, tag="tsum")
        nc.vector.tensor_scalar_add(ra1[:, 0:DVE_SLOTS], racc_d[:, :], 1.0)
        nc.vector.tensor_scalar_add(ra1[:, DVE_SLOTS:8], racc_a[:, :], 1.0)
        nc.vector.reciprocal(rainv[:, :], ra1[:, :])
        nc.vector.tensor_scalar_add(rp1[:, :], racc_p[:, :], 1.0)
        nc.vector.tensor_tensor(out=tq[:, :], in0=rp1[:, :], in1=rainv[:, :],
                                op=ALU.mult)
        nc.vector.reduce_sum(tsum[:, :], tq[:, :], axis=AX.X)
        ps_tot = ps.tile([1, 1], f32, tag="ps_tot")
        nc.tensor.matmul(ps_tot[:, :], onesc[:, :], tsum[:, :],
                         start=True, stop=True)
        out_sb = sb.tile([1, 1], f32, tag="out_sb")
        nc.scalar.copy(out_sb[:, :], ps_tot[:, :])
        nc.sync.dma_start(out_d.ap(), out_sb[:, :])
    nc.compile()
    return nc


_NC1 = None
_NC2 = None


def _get_ncs():
    global _NC1, _NC2
    if _NC1 is None:
        _NC1 = build_phase1()
        _NC2 = build_phase2()
    return _NC1, _NC2


def make_in_maps1(outputs, targets):
    outputs = np.ascontiguousarray(
        np.asarray(outputs, dtype=np.float32)).reshape(BATCH, FEAT, HW)
    targets = np.ascontiguousarray(
        np.asarray(targets, dtype=np.float32)).reshape(BATCH, FEAT, HW)
    return [
        {
            "x_out": np.ascontiguousarray(outputs[m * BPC:(m + 1) * BPC]),
            "x_tgt": np.ascontiguousarray(targets[m * BPC:(m + 1) * BPC]),
        }
        for m in range(NCORES)
    ]


def make_in_maps2(results1):
    fT_all = np.concatenate([results1[m]["fT"] for m in range(NCORES)], axis=2)
    fT_all = np.ascontiguousarray(fT_all)
    return [
        {
            "fT_all": fT_all,
            "fT_own": np.ascontiguousarray(results1[m]["fT"]),
            "sg": results1[m]["sg"],
        }
        for m in range(NCORES)
    ]


def finish(results2):
    total = sum(float(results2[m]["out"][0, 0]) for m in range(NCORES))
    return np.array(1.0 - total / (GROUP * B2), dtype=np.float32)


def kernel(outputs, targets):
    nc1, nc2 = _get_ncs()
    res1 = run_bass_kernel_spmd(nc1, make_in_maps1(outputs, targets),
                                core_ids=list(range(NCORES)))
    res2 = run_bass_kernel_spmd(nc2, make_in_maps2(res1.results),
                                core_ids=list(range(NCORES)))
    return finish(res2.results)


if __name__ == "__main__":
    import reference as ref
    inputs = ref.setup_inputs()
    actual = kernel(**{k: np.asarray(v) for k, v in inputs.items()})
    print("kernel result:", actual)


# revision 7
# speedup vs baseline: 1.5390x; 1.0832x over previous
"""APKDA loss (pool+normalize -> SmoothAP) as two distributed Bass launches on
8 TRN2 NeuronCores.

Math restructuring vs the reference:
  - Only the diagonal class-blocks of sim_all_rk are ever used, so per query q
    we need rank sums only over its 16 same-class columns j:
        r_all[q,j] = 1 + sum_k relu(S[q,k] - S[q,j])   (k over all 512 columns)
        r_pos[q,j] = 1 + sum_k relu(Sg[q,k] - Sg[q,j]) (k over the 16-group)
    with Sg the own-class block of S.  The eye-masks in the reference only
    kill k==j terms whose relu is 0 anyway.
  - L2-normalizing the hw-sum equals normalizing the hw-mean (scale cancels).

Sharding: batch-parallel.  Core m owns 4 classes = outputs[32m:32m+32] and
targets[32m:32m+32] (6.4MB of the 51.4MB input).

Phase 1 (memory-bound, ~HBM roofline): each core pools + normalizes its 64
f-rows and writes its f^T block as bf16 [4,128,32] per branch (feature-major).
Phase 2: the host concatenates the blocks into the full f^T (512KB bf16) in
reference row order, feeds it to every core plus that core's own 64 columns;
each core computes its own-class Gram (for Sg), its [64, 512] slice of S, the
rank sums, and a partial sum of r_pos/r_all.  Sg and S come from the same
bf16 operands so the k==j relu terms cancel exactly.  The host sums the 8
partial scalars: loss = 1 - total/(16*512).

A single-launch variant with an in-kernel AllGather measured 133.7us:
collectives on this runtime have a ~40-80us latency floor regardless of size,
so the f exchange goes through the host instead (two NEFF launches at ~13us
fixed overhead each).
"""

import numpy as np
import ml_dtypes

import concourse.bass as bass
import concourse.bacc as bacc
import concourse.mybir as mybir
import concourse.tile as tile
from concourse.bass_utils import run_bass_kernel_spmd

F32 = mybir.dt.float32
BF16 = mybir.dt.bfloat16
NCORES = 8
BATCH, FEAT, HW = 256, 512, 49
BPC = BATCH // NCORES          # 32 batch rows per branch per core
GROUP, B2 = 16, 512

# j-slot split of the 8 (two-j-per-instruction) rank iterations
DVE_SLOTS = 4                  # slots 0..3 on VectorE, 4..7 on ScalarE

# targets-branch pooling chunk widths (c_local units); last chunk small so the
# pooling tail after the final DMA is short
T_CHUNKS = [40, 40, 40, 8]
O_CHUNKS = [64, 64]


def build_phase1(dbg=None):
    """Pool + normalize + transpose; out: fT_o/fT_t bf16 [4,128,32] (d-major)."""
    nc = bacc.Bacc("TRN2", target_bir_lowering=False, debug=False,
                   num_devices=NCORES)
    f32 = F32
    AF = mybir.ActivationFunctionType
    ALU = mybir.AluOpType
    AX = mybir.AxisListType
    x_out = nc.dram_tensor("x_out", [BPC, FEAT, HW], f32, kind="ExternalInput")
    x_tgt = nc.dram_tensor("x_tgt", [BPC, FEAT, HW], f32, kind="ExternalInput")
    fto_d = nc.dram_tensor("fT_o", [4, 128, 32], BF16, kind="ExternalOutput")
    ftt_d = nc.dram_tensor("fT_t", [4, 128, 32], BF16, kind="ExternalOutput")
    ident_d = nc.inline_tensor(np.eye(128, dtype=np.float32), "ident_c")
    mbsum_d = nc.inline_tensor(
        np.tile(np.eye(32, dtype=np.float32), (4, 4)), "mbsum_c")

    with tile.TileContext(nc) as tc, (
            tc.tile_pool(name="sb", bufs=1)) as sb, (
            tc.tile_pool(name="ps", bufs=1, space="PSUM")) as ps:
        xo = sb.tile([128, 6272], f32, tag="xo")
        xt = sb.tile([128, 6272], f32, tag="xt")
        pooled_o = sb.tile([128, 128], f32, tag="pooled_o")
        pooled_t = sb.tile([128, 128], f32, tag="pooled_t")

        # warm the ACT table set early (Sqrt + Copy live in one set) so no
        # table load lands on the critical path later
        dummy = sb.tile([1, 2], f32, tag="dummy")
        nc.vector.memset(dummy[:, :], 1.0)
        nc.scalar.activation(dummy[:, 0:1], dummy[:, 1:2], AF.Sqrt)

        # Input DMAs.  partition p = 32g + b; row (g,b) holds x[b, 128g:128g+128, :]
        # flattened.  Outputs branch loads fully first (its normalize/transpose
        # then overlaps the targets load).  partitions 0-63 (g=0,1) ride the
        # sync HWDGE ring -> even SDMA engines; g=2,3 ride scalar -> odd.
        def load_chunks(t_, x_, p_, widths):
            c0 = 0
            for w in widths:
                for g in range(4):
                    eng = nc.sync if g < 2 else nc.scalar
                    eng.dma_start(
                        t_[32 * g:32 * (g + 1), 49 * c0:49 * (c0 + w)],
                        x_.ap()[:, g * 128 + c0:g * 128 + c0 + w, :])
                nc.vector.reduce_sum(
                    p_[:, c0:c0 + w],
                    t_[:, 49 * c0:49 * (c0 + w)].rearrange(
                        "p (c h) -> p c h", h=HW),
                    axis=AX.X)
                c0 += w

        load_chunks(xo, x_out, pooled_o, O_CHUNKS)
        load_chunks(xt, x_tgt, pooled_t, T_CHUNKS)

        # constants (queued on sync after input DMAs; small)
        ident = sb.tile([128, 128], f32, tag="ident")
        mbsum = sb.tile([128, 128], f32, tag="mbsum")
        nc.sync.dma_start(ident[:, :], ident_d.ap())
        nc.sync.dma_start(mbsum[:, :], mbsum_d.ap())

        scrap_n = sb.tile([128, 128], f32, tag="scrap_n")
        ps_n = ps.tile([128, 2], f32, tag="ps_n")
        nrm = sb.tile([128, 2], f32, tag="nrm")
        inv = sb.tile([128, 2], f32, tag="inv")
        r2 = sb.tile([128, 2], f32, tag="r2")

        def branch_tail(pooled, col, f_n, psT, out_bf, out_d):
            # r2 = sum_c pooled^2 (DVE, avoids an ACT Square table set)
            nc.vector.scalar_tensor_tensor(
                out=scrap_n[:, :], in0=pooled[:, :], scalar=0.0,
                in1=pooled[:, :], op0=ALU.add, op1=ALU.mult,
                accum_out=r2[:, col:col + 1])
            # cross-partition sum of the 4 g-groups per b, replicated
            nc.tensor.matmul(ps_n[:, col:col + 1], mbsum[:, :],
                             r2[:, col:col + 1], start=True, stop=True)
            nc.scalar.activation(nrm[:, col:col + 1], ps_n[:, col:col + 1],
                                 AF.Sqrt)
            nc.vector.reciprocal(inv[:, col:col + 1], nrm[:, col:col + 1])
            nc.vector.tensor_scalar_mul(f_n[:, :], pooled[:, :],
                                        inv[:, col:col + 1])
            nc.tensor.transpose(psT[:, :], f_n[:, :], ident[:, :])
            for g in range(4):
                cp = nc.vector.tensor_copy if g % 2 == 0 else nc.scalar.copy
                cp(out_bf[g][:, :], psT[:, 32 * g:32 * (g + 1)])
            for g in range(4):
                eng = nc.sync if g < 2 else nc.scalar
                eng.dma_start(out_d.ap()[g], out_bf[g][:, :])

        fo_n = sb.tile([128, 128], f32, tag="fo_n")
        ft_n = sb.tile([128, 128], f32, tag="ft_n")
        psT_o = ps.tile([128, 128], f32, tag="psT_o")
        psT_t = ps.tile([128, 128], f32, tag="psT_t")
        fto = [sb.tile([128, 32], BF16, tag=f"fto{g}", name=f"fto{g}")
               for g in range(4)]
        ftt = [sb.tile([128, 32], BF16, tag=f"ftt{g}", name=f"ftt{g}")
               for g in range(4)]
        branch_tail(pooled_o, 0, fo_n, psT_o, fto, fto_d)
        branch_tail(pooled_t, 1, ft_n, psT_t, ftt, ftt_d)
    nc.compile()
    return nc


def build_phase2(dbg=None):
    """S slice + rank sums from replicated bf16 f^T.
    in: fT_all [4,128,512], fT_own [4,128,64] (both bf16, reference col order);
    out: [1,1] f32 partial sum."""
    nc = bacc.Bacc("TRN2", target_bir_lowering=False, debug=False,
                   num_devices=NCORES)
    f32 = F32
    AF = mybir.ActivationFunctionType
    ALU = mybir.AluOpType
    AX = mybir.AxisListType
    fT_all = nc.dram_tensor("fT_all", [4, 128, 512], BF16, kind="ExternalInput")
    fT_own = nc.dram_tensor("fT_own", [4, 128, 64], BF16, kind="ExternalInput")
    out_d = nc.dram_tensor("out", [1, 1], f32, kind="ExternalOutput")
    ones_d = nc.inline_tensor(np.ones((128, 1), dtype=np.float32), "ones_c")

    with tile.TileContext(nc) as tc, (
            tc.tile_pool(name="sb", bufs=1)) as sb, (
            tc.tile_pool(name="ps", bufs=1, space="PSUM")) as ps:
        rhs = [sb.tile([128, 512], BF16, tag=f"rhs{g}", name=f"rhsg{g}")
               for g in range(4)]
        ccin = [sb.tile([128, 64], BF16, tag=f"ccin{g}", name=f"ccin{g}")
                for g in range(4)]
        # own block first so the Gram/Sg prep runs in the shadow of the big
        # load; even/odd partition halves ride the two HWDGE rings
        for g in range(4):
            nc.sync.dma_start(ccin[g][0:64, :], fT_own.ap()[g, 0:64, :])
            nc.scalar.dma_start(ccin[g][64:128, :], fT_own.ap()[g, 64:128, :])
        for g in range(4):
            nc.sync.dma_start(rhs[g][0:64, :], fT_all.ap()[g, 0:64, :])
            nc.scalar.dma_start(rhs[g][64:128, :], fT_all.ap()[g, 64:128, :])
        onesc = sb.tile([128, 1], f32, tag="onesc")
        nc.sync.dma_start(onesc[:, :], ones_d.ap())

        # own-class Gram -> Sg (the diagonal 16x16 blocks).  Compute engines
        # need 32-aligned partition bases, so stage in SBUF and extract the
        # diag blocks with small DMAs.
        ps_G = ps.tile([64, 64], f32, tag="ps_G")
        for g in range(4):
            nc.tensor.matmul(ps_G[:, :], ccin[g][:, :], ccin[g][:, :],
                             start=(g == 0), stop=(g == 3))
        G_sb = sb.tile([64, 64], f32, tag="G_sb")
        nc.vector.tensor_copy(G_sb[:, :], ps_G[:, :])
        Sg = sb.tile([64, 16], f32, tag="Sg")
        for c in range(4):
            nc.scalar.dma_start(Sg[16 * c:16 * (c + 1), :],
                                G_sb[16 * c:16 * (c + 1), 16 * c:16 * (c + 1)])
        negSg = sb.tile([64, 16], f32, tag="negSg")
        nc.vector.tensor_scalar_mul(negSg[:, :], Sg[:, :], -1.0)
        B8 = sb.tile([128, 8], f32, tag="B8")
        SgD = sb.tile([128, 16], f32, tag="SgD")
        nc.vector.tensor_copy(B8[0:64, :], negSg[:, 0:8])
        nc.vector.tensor_copy(B8[64:128, :], negSg[:, 8:16])
        nc.vector.tensor_copy(SgD[0:64, :], Sg[:, :])
        nc.vector.tensor_copy(SgD[64:128, :], Sg[:, :])
        zeros = sb.tile([128, 512], f32, tag="zeros")
        nc.vector.memset(zeros[:, :], 0.0)

        # S slice: [64 own queries, 512 keys], bf16 @ 1 cyc/row
        ps_S = ps.tile([64, 512], f32, tag="ps_S")
        for g in range(4):
            nc.tensor.matmul(ps_S[:, :], ccin[g][:, :], rhs[g][:, :],
                             start=(g == 0), stop=(g == 3))
        Sb = sb.tile([128, 512], f32, tag="Sb")
        nc.scalar.copy(Sb[0:64, :], ps_S[:, :])
        nc.vector.tensor_copy(Sb[64:128, :], ps_S[:, :])

        scrap_d = sb.tile([128, 512], f32, tag="scrap_d")
        scrap_a = sb.tile([128, 512], f32, tag="scrap_a")
        scrap_p = sb.tile([128, 16], f32, tag="scrap_p")
        racc_d = sb.tile([128, DVE_SLOTS], f32, tag="racc_d")
        racc_a = sb.tile([128, 8 - DVE_SLOTS], f32, tag="racc_a")
        racc_p = sb.tile([128, 8], f32, tag="racc_p")
        for i in range(8):
            if i < DVE_SLOTS:
                nc.vector.scalar_tensor_tensor(
                    out=scrap_d[:, :], in0=Sb[:, :], scalar=B8[:, i:i + 1],
                    in1=zeros[:, :], op0=ALU.add, op1=ALU.max,
                    accum_out=racc_d[:, i:i + 1])
            else:
                nc.scalar.activation(
                    scrap_a[:, :], Sb[:, :], AF.Relu, bias=B8[:, i:i + 1],
                    accum_out=racc_a[:, i - DVE_SLOTS:i - DVE_SLOTS + 1])
            nc.vector.scalar_tensor_tensor(
                out=scrap_p[:, :], in0=SgD[:, :], scalar=B8[:, i:i + 1],
                in1=zeros[:, 0:16], op0=ALU.add, op1=ALU.max,
                accum_out=racc_p[:, i:i + 1])

        # partial AP sum: sum over (q, j) of (1+r_pos)/(1+r_all)
        ra1 = sb.tile([128, 8], f32, tag="ra1")
        rainv = sb.tile([128, 8], f32, tag="rainv")
        rp1 = sb.tile([128, 8], f32, tag="rp1")
        tq = sb.tile([128, 8], f32, tag="tq")
        tsum = sb.tile([128, 1], f32, tag="tsum")
        nc.vector.tensor_scalar_add(ra1[:, 0:DVE_SLOTS], racc_d[:, :], 1.0)
        nc.vector.tensor_scalar_add(ra1[:, DVE_SLOTS:8], racc_a[:, :], 1.0)
        nc.vector.reciprocal(rainv[:, :], ra1[:, :])
        nc.vector.tensor_scalar_add(rp1[:, :], racc_p[:, :], 1.0)
        nc.vector.tensor_tensor(out=tq[:, :], in0=rp1[:, :], in1=rainv[:, :],
                                op=ALU.mult)
        nc.vector.reduce_sum(tsum[:, :], tq[:, :], axis=AX.X)
        ps_tot = ps.tile([1, 1], f32, tag="ps_tot")
        nc.tensor.matmul(ps_tot[:, :], onesc[:, :], tsum[:, :],
                         start=True, stop=True)
        out_sb = sb.tile([1, 1], f32, tag="out_sb")
        nc.scalar.copy(out_sb[:, :], ps_tot[:, :])
        nc.sync.dma_start(out_d.ap(), out_sb[:, :])
    nc.compile()
    return nc


_NC1 = None
_NC2 = None


def _get_ncs():
    global _NC1, _NC2
    if _NC1 is None:
        _NC1 = build_phase1()
        _NC2 = build_phase2()
    return _NC1, _NC2


def make_in_maps1(outputs, targets):
    outputs = np.ascontiguousarray(
        np.asarray(outputs, dtype=np.float32)).reshape(BATCH, FEAT, HW)
    targets = np.ascontiguousarray(
        np.asarray(targets, dtype=np.float32)).reshape(BATCH, FEAT, HW)
    return [
        {
            "x_out": np.ascontiguousarray(outputs[m * BPC:(m + 1) * BPC]),
            "x_tgt": np.ascontiguousarray(targets[m * BPC:(m + 1) * BPC]),
        }
        for m in range(NCORES)
    ]


# column permutation: branch-ordered [out b, tgt b] -> reference interleaved
# col = 16*(b//8) + 8*branch + b%8
_PERM = np.empty(64, np.int64)
for _b in range(32):
    _PERM[16 * (_b // 8) + (_b % 8)] = _b            # outputs branch
    _PERM[16 * (_b // 8) + 8 + (_b % 8)] = 32 + _b   # targets branch


def make_in_maps2(results1):
    # per-core interleaved block [4, 128, 64], then concat to [4, 128, 512]
    blocks = []
    for m in range(NCORES):
        both = np.concatenate([results1[m]["fT_o"], results1[m]["fT_t"]],
                              axis=2)  # [4,128,64] cols = [out b | tgt b]
        blocks.append(np.ascontiguousarray(both[:, :, _PERM]))
    fT_all = np.ascontiguousarray(np.concatenate(blocks, axis=2))
    return [
        {"fT_all": fT_all, "fT_own": blocks[m]}
        for m in range(NCORES)
    ]


def finish(results2):
    total = sum(float(results2[m]["out"][0, 0]) for m in range(NCORES))
    return np.array(1.0 - total / (GROUP * B2), dtype=np.float32)


def kernel(outputs, targets):
    nc1, nc2 = _get_ncs()
    res1 = run_bass_kernel_spmd(nc1, make_in_maps1(outputs, targets),
                                core_ids=list(range(NCORES)))
    res2 = run_bass_kernel_spmd(nc2, make_in_maps2(res1.results),
                                core_ids=list(range(NCORES)))
    return finish(res2.results)


if __name__ == "__main__":
    import reference as ref
    inputs = ref.setup_inputs()
    actual = kernel(**{k: np.asarray(v) for k, v in inputs.items()})
    print("kernel result:", actual)


# revision 9
# speedup vs baseline: 1.7566x; 1.1414x over previous
"""APKDA loss (pool+normalize -> SmoothAP) as two distributed Bass launches on
8 TRN2 NeuronCores.

Math restructuring vs the reference:
  - Only the diagonal class-blocks of sim_all_rk are ever used, so per query q
    we need rank sums only over its 16 same-class columns j:
        r_all[q,j] = 1 + sum_k relu(S[q,k] - S[q,j])   (k over all 512 columns)
        r_pos[q,j] = 1 + sum_k relu(Sg[q,k] - Sg[q,j]) (k over the 16-group)
    with Sg the own-class block of S.  The eye-masks in the reference only
    kill k==j terms whose relu is 0 anyway.
  - L2-normalizing the hw-sum equals normalizing the hw-mean (scale cancels).

Sharding: batch-parallel.  Core m owns 4 classes = outputs[32m:32m+32] and
targets[32m:32m+32] (6.4MB of the 51.4MB input).

Phase 1 (memory-bound): each core sum-pools its 6.4MB shard over the 7x7
window and returns the raw [128(g,b), 128] pooled sums per branch as bf16.
Host: normalize rows, transpose to feature-major, interleave to reference row
order, concatenate all cores -> full f^T (512KB bf16).
Phase 2: every core gets the full f^T plus its own 64 columns, computes the
own-class Gram (for Sg), its [64, 512] slice of S, and the raw rank sums;
host applies the +1/division/total.  Sg and S come from the same bf16
operands and identical PE accumulation, so the k==j relu terms cancel
exactly.

A single-launch variant with an in-kernel AllGather measured 133.7us:
collectives on this runtime have a ~40-80us latency floor regardless of size,
so the f exchange goes through the host instead (two NEFF launches at ~13us
fixed overhead each).  Input DMA tops out at ~205 GB/s/core here no matter
the descriptor path (sync/scalar/gpsimd) or shape, so phase 1 is pinned at
~31us of DMA + overheads.
"""

import numpy as np
import ml_dtypes

import concourse.bass as bass
import concourse.bacc as bacc
import concourse.mybir as mybir
import concourse.tile as tile
from concourse.bass_utils import run_bass_kernel_spmd

F32 = mybir.dt.float32
BF16 = mybir.dt.bfloat16
NCORES = 8
BATCH, FEAT, HW = 256, 512, 49
BPC = BATCH // NCORES          # 32 batch rows per branch per core
GROUP, B2 = 16, 512

# j-slot split of the 8 (two-j-per-instruction) rank iterations
DVE_SLOTS = 4                  # slots 0..3 on VectorE, 4..7 on ScalarE

# pooling chunk widths (c_local units); outputs loads first, targets' last
# chunk is small so the pooling tail after the final DMA is short
O_CHUNKS = [64, 64]
T_CHUNKS = [48, 48, 24, 8]


def build_phase1(dbg=None):
    """Sum-pool the shard; out: p_o / p_t bf16 [128(g,b), 128 c_local]."""
    nc = bacc.Bacc("TRN2", target_bir_lowering=False, debug=False,
                   num_devices=NCORES)
    f32 = F32
    AX = mybir.AxisListType
    x_out = nc.dram_tensor("x_out", [BPC, FEAT, HW], f32, kind="ExternalInput")
    x_tgt = nc.dram_tensor("x_tgt", [BPC, FEAT, HW], f32, kind="ExternalInput")
    po_d = nc.dram_tensor("p_o", [128, 128], BF16, kind="ExternalOutput")
    pt_d = nc.dram_tensor("p_t", [128, 128], BF16, kind="ExternalOutput")

    with tile.TileContext(nc) as tc, tc.tile_pool(name="sb", bufs=1) as sb:
        xo = sb.tile([128, 6272], f32, tag="xo")
        xt = sb.tile([128, 6272], f32, tag="xt")
        pooled_o = sb.tile([128, 128], f32, tag="pooled_o")
        pooled_t = sb.tile([128, 128], f32, tag="pooled_t")
        po_bf = sb.tile([128, 128], BF16, tag="po_bf")
        pt_bf = sb.tile([128, 128], BF16, tag="pt_bf")

        # partition p = 32g + b; row (g,b) holds x[b, 128g:128g+128, :] flat.
        # g=0,1 (partitions 0-63) ride the sync HWDGE ring, g=2,3 ride scalar,
        # which spreads the load over all 16 SDMA engines.
        def load_chunks(t_, x_, p_, widths):
            c0 = 0
            for w in widths:
                for g in range(4):
                    eng = nc.sync if g < 2 else nc.scalar
                    eng.dma_start(
                        t_[32 * g:32 * (g + 1), 49 * c0:49 * (c0 + w)],
                        x_.ap()[:, g * 128 + c0:g * 128 + c0 + w, :])
                nc.vector.reduce_sum(
                    p_[:, c0:c0 + w],
                    t_[:, 49 * c0:49 * (c0 + w)].rearrange(
                        "p (c h) -> p c h", h=HW),
                    axis=AX.X)
                c0 += w

        load_chunks(xo, x_out, pooled_o, O_CHUNKS)
        load_chunks(xt, x_tgt, pooled_t, T_CHUNKS)

        nc.vector.tensor_copy(po_bf[:, :], pooled_o[:, :])
        nc.sync.dma_start(po_d.ap()[0:64, :], po_bf[0:64, :])
        nc.scalar.dma_start(po_d.ap()[64:128, :], po_bf[64:128, :])
        nc.vector.tensor_copy(pt_bf[:, :], pooled_t[:, :])
        nc.sync.dma_start(pt_d.ap()[0:64, :], pt_bf[0:64, :])
        nc.scalar.dma_start(pt_d.ap()[64:128, :], pt_bf[64:128, :])
    nc.compile()
    return nc


def build_phase2(dbg=None):
    """S slice + raw rank sums from replicated bf16 f^T.
    in: fT_all [4,128,512], fT_own [4,128,64] (bf16, reference col order);
    out: racc [128, 16] f32 (cols 0-7 r_all slots, 8-15 r_pos slots)."""
    nc = bacc.Bacc("TRN2", target_bir_lowering=False, debug=False,
                   num_devices=NCORES)
    f32 = F32
    AF = mybir.ActivationFunctionType
    ALU = mybir.AluOpType
    fT_all = nc.dram_tensor("fT_all", [4, 128, 512], BF16, kind="ExternalInput")
    fT_own = nc.dram_tensor("fT_own", [4, 128, 64], BF16, kind="ExternalInput")
    out_d = nc.dram_tensor("out", [128, 16], f32, kind="ExternalOutput")

    with tile.TileContext(nc) as tc, (
            tc.tile_pool(name="sb", bufs=1)) as sb, (
            tc.tile_pool(name="ps", bufs=1, space="PSUM")) as ps:
        ccin = sb.tile([128, 256], BF16, tag="ccin")   # free = (g, col)
        rhs = sb.tile([128, 2048], BF16, tag="rhs")    # free = (g, key)
        # own block first so the Gram/Sg prep runs in the shadow of the big
        # load; even/odd partition halves ride the two HWDGE rings
        nc.sync.dma_start(
            ccin[0:64, :].rearrange("p (g n) -> p g n", g=4),
            fT_own.ap()[:, 0:64, :].rearrange("g p n -> p g n"))
        nc.scalar.dma_start(
            ccin[64:128, :].rearrange("p (g n) -> p g n", g=4),
            fT_own.ap()[:, 64:128, :].rearrange("g p n -> p g n"))
        nc.sync.dma_start(
            rhs[0:64, :].rearrange("p (g n) -> p g n", g=4),
            fT_all.ap()[:, 0:64, :].rearrange("g p n -> p g n"))
        nc.scalar.dma_start(
            rhs[64:128, :].rearrange("p (g n) -> p g n", g=4),
            fT_all.ap()[:, 64:128, :].rearrange("g p n -> p g n"))

        def lhsT(g):
            return ccin[:, 64 * g:64 * (g + 1)]

        # own-class Gram -> Sg (the diagonal 16x16 blocks).  Compute engines
        # need 32-aligned partition bases, so stage in SBUF and extract the
        # diag blocks with small DMAs.
        ps_G = ps.tile([64, 64], f32, tag="ps_G")
        for g in range(4):
            nc.tensor.matmul(ps_G[:, :], lhsT(g), lhsT(g),
                             start=(g == 0), stop=(g == 3))
        G_sb = sb.tile([64, 64], f32, tag="G_sb")
        nc.vector.tensor_copy(G_sb[:, :], ps_G[:, :])
        Sg = sb.tile([64, 16], f32, tag="Sg")
        for c in range(4):
            nc.scalar.dma_start(Sg[16 * c:16 * (c + 1), :],
                                G_sb[16 * c:16 * (c + 1), 16 * c:16 * (c + 1)])
        negSg = sb.tile([64, 16], f32, tag="negSg")
        nc.vector.tensor_scalar_mul(negSg[:, :], Sg[:, :], -1.0)
        B8 = sb.tile([128, 8], f32, tag="B8")
        SgD = sb.tile([128, 16], f32, tag="SgD")
        nc.vector.tensor_copy(B8[0:64, :], negSg[:, 0:8])
        nc.vector.tensor_copy(B8[64:128, :], negSg[:, 8:16])
        nc.vector.tensor_copy(SgD[0:64, :], Sg[:, :])
        nc.vector.tensor_copy(SgD[64:128, :], Sg[:, :])
        zeros = sb.tile([128, 512], f32, tag="zeros")
        nc.vector.memset(zeros[:, :], 0.0)

        # S slice: [64 own queries, 512 keys], bf16 @ 1 cyc/row
        ps_S = ps.tile([64, 512], f32, tag="ps_S")
        for g in range(4):
            nc.tensor.matmul(ps_S[:, :], lhsT(g), rhs[:, 512 * g:512 * (g + 1)],
                             start=(g == 0), stop=(g == 3))
        Sb = sb.tile([128, 512], f32, tag="Sb")
        nc.scalar.copy(Sb[0:64, :], ps_S[:, :])
        nc.vector.tensor_copy(Sb[64:128, :], ps_S[:, :])

        scrap_d = sb.tile([128, 512], f32, tag="scrap_d")
        scrap_a = sb.tile([128, 512], f32, tag="scrap_a")
        scrap_p = sb.tile([128, 16], f32, tag="scrap_p")
        racc = sb.tile([128, 16], f32, tag="racc")
        for i in range(8):
            # r_pos first: it only needs SgD, so DVE starts before S lands
            nc.vector.scalar_tensor_tensor(
                out=scrap_p[:, :], in0=SgD[:, :], scalar=B8[:, i:i + 1],
                in1=zeros[:, 0:16], op0=ALU.add, op1=ALU.max,
                accum_out=racc[:, 8 + i:9 + i])
        for i in range(8):
            if i < DVE_SLOTS:
                nc.vector.scalar_tensor_tensor(
                    out=scrap_d[:, :], in0=Sb[:, :], scalar=B8[:, i:i + 1],
                    in1=zeros[:, :], op0=ALU.add, op1=ALU.max,
                    accum_out=racc[:, i:i + 1])
            else:
                nc.scalar.activation(
                    scrap_a[:, :], Sb[:, :], AF.Relu, bias=B8[:, i:i + 1],
                    accum_out=racc[:, i:i + 1])
        nc.sync.dma_start(out_d.ap()[0:64, :], racc[0:64, :])
        nc.scalar.dma_start(out_d.ap()[64:128, :], racc[64:128, :])
    nc.compile()
    return nc


_NC1 = None
_NC2 = None


def _get_ncs():
    global _NC1, _NC2
    if _NC1 is None:
        _NC1 = build_phase1()
        _NC2 = build_phase2()
    return _NC1, _NC2


def make_in_maps1(outputs, targets):
    outputs = np.ascontiguousarray(
        np.asarray(outputs, dtype=np.float32)).reshape(BATCH, FEAT, HW)
    targets = np.ascontiguousarray(
        np.asarray(targets, dtype=np.float32)).reshape(BATCH, FEAT, HW)
    return [
        {
            "x_out": np.ascontiguousarray(outputs[m * BPC:(m + 1) * BPC]),
            "x_tgt": np.ascontiguousarray(targets[m * BPC:(m + 1) * BPC]),
        }
        for m in range(NCORES)
    ]


# column permutation: branch-ordered [out b, tgt b] -> reference interleaved
# col = 16*(b//8) + 8*branch + b%8
_PERM = np.empty(64, np.int64)
for _b in range(32):
    _PERM[16 * (_b // 8) + (_b % 8)] = _b            # outputs branch
    _PERM[16 * (_b // 8) + 8 + (_b % 8)] = 32 + _b   # targets branch


def make_in_maps2(results1):
    """pooled [128(g,b), 128] bf16 per branch -> bf16 fT blocks, interleaved."""
    blocks = []
    for m in range(NCORES):
        fs = []
        for key in ("p_o", "p_t"):
            p = results1[m][key].astype(np.float32)       # [128, 128]
            v = np.concatenate([p[32 * g:32 * (g + 1), :] for g in range(4)],
                               axis=1)                    # [32 b, 512 c]
            fs.append(v / np.linalg.norm(v, axis=1, keepdims=True))
        f = np.concatenate(fs, axis=0)                    # [64 rows, 512]
        f = f[_PERM, :]                                   # reference order
        fT = f.T.reshape(4, 128, 64)                      # [g, d_local, col]
        blocks.append(fT.astype(ml_dtypes.bfloat16))
    fT_all = np.ascontiguousarray(np.concatenate(blocks, axis=2))
    return [{"fT_all": fT_all, "fT_own": np.ascontiguousarray(blocks[m])}
            for m in range(NCORES)]


def finish(results2):
    total = 0.0
    for m in range(NCORES):
        racc = results2[m]["out"].astype(np.float64)      # [128, 16]
        total += ((1.0 + racc[:, 8:16]) / (1.0 + racc[:, 0:8])).sum()
    return np.array(1.0 - total / (GROUP * B2), dtype=np.float32)


def kernel(outputs, targets):
    nc1, nc2 = _get_ncs()
    res1 = run_bass_kernel_spmd(nc1, make_in_maps1(outputs, targets),
                                core_ids=list(range(NCORES)))
    res2 = run_bass_kernel_spmd(nc2, make_in_maps2(res1.results),
                                core_ids=list(range(NCORES)))
    return finish(res2.results)


if __name__ == "__main__":
    import reference as ref
    inputs = ref.setup_inputs()
    actual = kernel(**{k: np.asarray(v) for k, v in inputs.items()})
    print("kernel result:", actual)
